# revision 32
# baseline (speedup 1.0000x reference)
# Trainium2 Bass kernel for AttentionWithSink
# B=2, S=2048, D=1024, H=16 heads (hd=64), 8 sink tokens, full bidirectional
# attention over T=2056 tokens, output projection back to D.
#
# Sharding: 8 cores = 2 batches x 4 head-groups (4 heads each).
# Each core computes QKV for its 4 heads over its batch, transposed-scores
# attention (keys on partitions => no transposes anywhere), and a partial
# output projection over its 256 head-dims. Host sums the 4 partials per
# batch (tensor-parallel unshard).
#
# The kernel is scalar(exp)-bound: 136 EXP activations of [*,1024] are the
# critical path. Everything else is software-pipelined around a gapless
# exp stream:
#  - all-bf16 data path (fp32 PSUM accumulate), few big input DMAs
#  - attention runs in 8 segments (qc x pr); attn@V lags S/exp by one
#    segment (PT tiles buffered in SBUF) so the PE never blocks scalar
#  - v' blocks are [ones|vA|ones|vB]: the attn@V matmul replicates the
#    softmax denominator across 64 partitions, so normalization is one
#    reciprocal_approx_fast + one multiply per head (no broadcasts)
#  - v-projection / q1-3 projections / output-projection units drain from
#    a backlog into the PE slack between score matmuls
import numpy as np

B, S, D, H, HD, NS = 2, 2048, 1024, 16, 64, 8
T = S + NS            # 2056 tokens incl. sinks (sinks stored LAST)
NCORES = 8
HPG = 4               # heads per group/core
GD = HPG * HD         # 256 head-dims per core
NKC = 17              # key chunks: 16*128 + 8
KREM = T - 16 * 128   # 8
NQC = 4               # query chunks
QCH = 512
VBLK = 256            # v' block: onesA(64)|vA(64)|onesB(64)|vB(64)
NSEG = 8

_prog_cache = {}


def _emit_body(nc, tc, tile, mybir, dr, pers, parts="baevdy"):
    from collections import deque

    BF16 = mybir.dt.bfloat16
    F32 = mybir.dt.float32
    AF = mybir.ActivationFunctionType
    ALU = mybir.AluOpType
    qT, kT, vp, wo_sb, bqt, bkt, bv_bc, bo_bc = pers

    with tc.tile_pool(name="xw", bufs=1) as xw:
        xt = [xw.tile([128, T], BF16, tag=f"x{dd}", name=f"x{dd}") for dd in range(8)]
        w_sb = {nm: xw.tile([128, 8 * GD], BF16, tag=f"w{nm}", name=f"w{nm}")
                for nm in ("q", "k", "v")}
        # DMA engines multiplex every in-flight transfer, so the only way to
        # prioritize the k-pass1 inputs (wk + first column-halves of x) is to
        # hold everything else back: each second-half transfer overlaps its
        # first half by 8 columns, so the WAW dependency keeps it queued until
        # that row-block's first half has fully landed; wq lands behind the
        # halves. The caller issued biases/wo already.
        XH = 2 * QCH       # 1024: chunks 0,1 in the first half
        nc.scalar.dma_start(w_sb["k"][:], dr["wk_t"][:])
        _eng = [nc.sync, nc.gpsimd, nc.scalar]
        for dd in range(8):
            for rh in range(2):
                _eng[(2 * dd + rh) % 3].dma_start(
                    xt[dd][rh * 64 : (rh + 1) * 64, 0:XH],
                    dr["xT"][dd * 128 + rh * 64 : dd * 128 + (rh + 1) * 64, 0:XH])
        nc.gpsimd.dma_start(w_sb["q"][:], dr["wq_t"][:])
        nc.scalar.dma_start(w_sb["v"][:], dr["wv_t"][:])
        for dd in range(8):
            _eng[(dd + 1) % 3].dma_start(
                xt[dd][:, XH:T], dr["xT"][dd * 128 : (dd + 1) * 128, XH:T])

        if "a" not in parts and "b" in parts:
          with tc.tile_pool(name="psk", bufs=5, space="PSUM") as psk:
            for i in range(2):
                for c5 in range(5):
                    n = QCH if c5 < 4 else KREM
                    pk = psk.tile([128, QCH], F32, tag="pk", name=f"pk{i}_{c5}")
                    for dd in range(8):
                        nc.tensor.matmul(
                            pk[:, :n],
                            w_sb["k"][:, dd * GD + i * 128 : dd * GD + i * 128 + 128],
                            xt[dd][:, c5 * QCH : c5 * QCH + n],
                            start=(dd == 0), stop=(dd == 7),
                        )
                    with nc.allow_low_precision(reason="bf16 kT within tol"):
                        nc.vector.tensor_scalar_add(
                            kT[i][:, c5 * QCH : c5 * QCH + n], pk[:, :n],
                            bkt[i][:, 0:1])
          return

        # ---------------- attention pipeline ---------------------------
        with (
            tc.tile_pool(name="sc", bufs=2, space="PSUM") as scp,
            tc.tile_pool(name="av", bufs=3, space="PSUM") as avp,
            tc.tile_pool(name="py", bufs=1, space="PSUM") as pyp,
            tc.tile_pool(name="pt", bufs=22) as ptp,
            tc.tile_pool(name="onT", bufs=8) as onp,
            tc.tile_pool(name="ysb", bufs=3) as yp,
            tc.tile_pool(name="small", bufs=4) as sp,
        ):
            backlog = deque()
            pts = {}     # seg -> list of PT tiles
            vts = {}     # seg -> (VA, VB)
            onts = {}    # seg -> onT

            def q_proj(qc, i):
                def go():
                    pq = pyp.tile([128, QCH], F32, tag="py", name=f"pq{i}_{qc}")
                    for dd in range(8):
                        nc.tensor.matmul(
                            pq[:],
                            w_sb["q"][:, dd * GD + i * 128 : dd * GD + i * 128 + 128],
                            xt[dd][:, qc * QCH : (qc + 1) * QCH],
                            start=(dd == 0), stop=(dd == 7),
                        )
                    with nc.allow_low_precision(reason="bf16 qT within tol"):
                        nc.vector.tensor_scalar_add(
                            qT[i][:, qc * QCH : (qc + 1) * QCH], pq[:],
                            bqt[i][:, 0:1])
                return go

            def k_proj_i1(c5):
                def go():
                    n = QCH if c5 < 4 else KREM
                    pk = pyp.tile([128, QCH], F32, tag="py", name=f"pk1_{c5}")
                    for dd in range(8):
                        nc.tensor.matmul(
                            pk[:, :n],
                            w_sb["k"][:, dd * GD + 128 : dd * GD + 256],
                            xt[dd][:, c5 * QCH : c5 * QCH + n],
                            start=(dd == 0), stop=(dd == 7),
                        )
                    with nc.allow_low_precision(reason="bf16 kT within tol"):
                        nc.vector.tensor_scalar_add(
                            kT[1][:, c5 * QCH : c5 * QCH + n], pk[:, :n],
                            bkt[1][:, 0:1])
                return go

            def v_chunk(tcx):
                def go():
                    kk = 128 if tcx < 16 else KREM
                    pv = pyp.tile([128, QCH], F32, tag="py", name=f"pv{tcx}")
                    for dd in range(8):
                        nc.tensor.matmul(
                            pv[:kk, 0:GD],
                            xt[dd][:, tcx * 128 : tcx * 128 + kk],
                            w_sb["v"][:, dd * GD : (dd + 1) * GD],
                            start=(dd == 0), stop=(dd == 7),
                        )
                    c0 = tcx * VBLK
                    for pr in range(2):
                        for hh in range(2):
                            h = pr * 2 + hh
                            with nc.allow_low_precision(reason="bf16 v within tol"):
                                nc.vector.tensor_tensor(
                                    vp[pr][:kk, c0 + 64 + hh * 128 : c0 + 128 + hh * 128],
                                    pv[:kk, h * 64 : (h + 1) * 64],
                                    bv_bc[:kk, h * 64 : (h + 1) * 64],
                                    op=ALU.add,
                                )
                return go

            def k_i0_pass(c5s):
                # dd-outer so each matmul chases its x block's DMA arrival;
                # psum tiles from the avp pool (free until seg1's attn@V)
                pks = {c5: avp.tile([128, QCH], F32, tag="av", name=f"pk0_{c5}")
                       for c5 in c5s}
                for dd in range(8):
                    for c5 in c5s:
                        n = QCH if c5 < 4 else KREM
                        nc.tensor.matmul(
                            pks[c5][:, :n],
                            w_sb["k"][:, dd * GD : dd * GD + 128],
                            xt[dd][:, c5 * QCH : c5 * QCH + n],
                            start=(dd == 0), stop=(dd == 7),
                        )
                for c5 in c5s:
                    n = QCH if c5 < 4 else KREM
                    with nc.allow_low_precision(reason="bf16 kT within tol"):
                        nc.vector.tensor_scalar_add(
                            kT[0][:, c5 * QCH : c5 * QCH + n], pks[c5][:, :n],
                            bkt[0][:, 0:1])

            def y_unit(qc, ts_, dc, drain=False):
                def go():
                    if drain:   # scp banks are free once the last exp is done
                        py = scp.tile([128, 2 * QCH], F32, tag="s",
                                      name=f"py_{qc}_{ts_}_{dc}")[:, 0:QCH]
                    else:
                        py = pyp.tile([128, QCH], F32, tag="py",
                                      name=f"py_{qc}_{ts_}_{dc}")
                    for pr in range(2):
                        nc.tensor.matmul(
                            py[:], onts[2 * qc + pr][:, ts_ * 128 : (ts_ + 1) * 128],
                            wo_sb[pr][:, dc * QCH : (dc + 1) * QCH],
                            start=(pr == 0), stop=(pr == 1),
                        )
                    ys = yp.tile([128, QCH], BF16, tag="ys", name=f"ys_{qc}_{ts_}_{dc}")
                    with nc.allow_low_precision(reason="bf16 y partials within tol"):
                        nc.vector.tensor_tensor(
                            ys[:], py[:], bo_bc[:, dc * QCH : (dc + 1) * QCH], op=ALU.add)
                    yeng = nc.sync if (ts_ + dc) % 2 == 0 else nc.gpsimd
                    yeng.dma_start(
                        dr["y"][qc * QCH + ts_ * 128 : qc * QCH + (ts_ + 1) * 128,
                                dc * QCH : (dc + 1) * QCH],
                        ys[:])
                return go

            def attnv(seg, kc):
                qc, pr = divmod(seg, 2)
                kk = 128 if kc < 16 else KREM
                if kc == 0:
                    vts[seg] = (
                        avp.tile([128, QCH], F32, tag="av", name=f"VA_{seg}"),
                        avp.tile([128, QCH], F32, tag="av", name=f"VB_{seg}"),
                    )
                VA, VB = vts[seg]
                PT = pts[seg][kc]
                c0 = kc * VBLK
                st, stp = kc == 0, kc == NKC - 1
                nc.tensor.matmul(
                    VA[:], vp[pr][:kk, c0 : c0 + 128],
                    PT[:kk, 0:QCH], start=st, stop=stp)
                nc.tensor.matmul(
                    VB[:], vp[pr][:kk, c0 + 128 : c0 + 256],
                    PT[:kk, QCH : 2 * QCH], start=st, stop=stp)

            def normalize(seg):
                VA, VB = vts[seg]
                onT = onp.tile([128, QCH], BF16, tag="onT", name=f"onT_{seg}")
                for hh, V in ((0, VA), (1, VB)):
                    if "d" in parts:
                        rc = sp.tile([64, QCH], F32, tag="rc", name=f"rc_{seg}_{hh}")
                        nc.vector.reciprocal_approx_fast(rc[:], V[0:64, :])
                        with nc.allow_low_precision(reason="bf16 attn out within tol"):
                            nc.vector.tensor_tensor(
                                onT[hh * 64 : hh * 64 + 64, :],
                                V[64:128, :], rc[:], op=ALU.mult)
                    else:
                        with nc.allow_low_precision(reason="bf16 attn out within tol"):
                            nc.vector.tensor_copy(
                                onT[hh * 64 : hh * 64 + 64, :], V[64:128, :])
                onts[seg] = onT
                del vts[seg]

            do_v = "v" in parts
            do_y = "y" in parts and do_v
            # v-projection first in the backlog (vp chunks needed by seg0's
            # attn@V, consumed from seg1 on)
            # Backlog order tracks x-DMA arrival: items needing only the
            # first column-half (k_i1 c5 0-1, qT[1], v chunks 0-7) come
            # first; second-half items late enough that the DMA has landed.
            if "b" in parts:
                backlog.append(k_proj_i1(0))
                backlog.append(k_proj_i1(1))
                backlog.append(q_proj(0, 1))
            if do_v:
                for tcx in range(8):
                    backlog.append(v_chunk(tcx))
            if "b" in parts:
                for c5 in (2, 3, 4):
                    backlog.append(k_proj_i1(c5))
            if do_v:
                for tcx in range(8, NKC):
                    backlog.append(v_chunk(tcx))

            for seg in range(NSEG):
                qc, pr = divmod(seg, 2)
                if seg in (1, 3, 5) and "b" in parts:
                    backlog.appendleft(q_proj(qc + 1, 1))
                    backlog.appendleft(q_proj(qc + 1, 0))
                if do_y and seg >= 3 and seg % 2 == 1:
                    yq = seg // 2 - 1   # y(qc0) at seg3 ... y(qc2) at seg7
                    for u in range(8):
                        backlog.append(y_unit(yq, u // 2, u % 2))
                pts[seg] = []
                for kc in range(NKC):
                    kk = 128 if kc < 16 else KREM
                    if seg == 0 and "b" in parts and kc in (0, 8):
                        if kc == 0:
                            k_i0_pass((0, 1))
                            q_proj(0, 0)()
                        else:
                            k_i0_pass((2, 3, 4))
                    Sps = scp.tile([128, 2 * QCH], F32, tag="s", name=f"S_{seg}_{kc}")
                    PT = ptp.tile([128, 2 * QCH], BF16, tag="pt", name=f"PT_{seg}_{kc}")
                    nc.tensor.matmul(
                        Sps[:kk, 0:QCH],
                        kT[pr][0:64, kc * 128 : kc * 128 + kk],
                        qT[pr][0:64, qc * QCH : (qc + 1) * QCH],
                        start=True, stop=True)
                    nc.tensor.matmul(
                        Sps[:kk, QCH : 2 * QCH],
                        kT[pr][64:128, kc * 128 : kc * 128 + kk],
                        qT[pr][64:128, qc * QCH : (qc + 1) * QCH],
                        start=True, stop=True)
                    if "e" in parts:
                        nc.scalar.activation(PT[:kk, :], Sps[:kk, :], AF.Exp)
                    pts[seg].append(PT)
                    if do_v and seg >= 1:
                        attnv(seg - 1, kc)
                    if backlog:
                        backlog.popleft()()
                if do_v and seg >= 1:
                    normalize(seg - 1)
                    del pts[seg - 1]

            # drain: attn@V for the last segment, leftovers, y(qc3). The
            # first three y units start their onT(seg6) matmul before
            # normalize(seg7) lands so the PE streams through its latency.
            if do_v:
                for kc in range(NKC):
                    attnv(NSEG - 1, kc)
                    if backlog:
                        backlog.popleft()()
                open_py = []
                if do_y:
                    for u in range(3):
                        ts_, dc = u // 2, u % 2
                        py = scp.tile([128, 2 * QCH], F32, tag="s",
                                      name=f"pyd_{ts_}_{dc}")[:, 0:QCH] \
                            if u < 2 else pyp.tile([128, QCH], F32, tag="py",
                                                   name=f"pyd_{ts_}_{dc}")
                        nc.tensor.matmul(
                            py[:], onts[6][:, ts_ * 128 : (ts_ + 1) * 128],
                            wo_sb[0][:, dc * QCH : (dc + 1) * QCH],
                            start=True, stop=False)
                        open_py.append((py, ts_, dc))
                normalize(NSEG - 1)
                for py, ts_, dc in open_py:
                    nc.tensor.matmul(
                        py[:], onts[7][:, ts_ * 128 : (ts_ + 1) * 128],
                        wo_sb[1][:, dc * QCH : (dc + 1) * QCH],
                        start=False, stop=True)
                    ys = yp.tile([128, QCH], BF16, tag="ys", name=f"ysd_{ts_}_{dc}")
                    with nc.allow_low_precision(reason="bf16 y partials within tol"):
                        nc.vector.tensor_tensor(
                            ys[:], py[:], bo_bc[:, dc * QCH : (dc + 1) * QCH], op=ALU.add)
                    yeng = nc.sync if (ts_ + dc) % 2 == 0 else nc.gpsimd
                    yeng.dma_start(
                        dr["y"][3 * QCH + ts_ * 128 : 3 * QCH + (ts_ + 1) * 128,
                                dc * QCH : (dc + 1) * QCH],
                        ys[:])
            while backlog:
                backlog.popleft()()
            if do_y:
                for u in range(3, 8):
                    y_unit(3, u // 2, u % 2, drain=True)()


def _build_program(reps=1, parts="baevdy"):
    import concourse.bass as bass  # noqa: F401
    import concourse.mybir as mybir
    import concourse.tile as tile
    from concourse import bacc

    BF16 = mybir.dt.bfloat16
    F32 = mybir.dt.float32

    nc = bacc.Bacc("TRN2", num_devices=NCORES)
    dr = {
        "xT": nc.dram_tensor("xT", [D, T], BF16, kind="ExternalInput"),
        "wq_t": nc.dram_tensor("wq_t", [128, 8 * GD], BF16, kind="ExternalInput"),
        "wk_t": nc.dram_tensor("wk_t", [128, 8 * GD], BF16, kind="ExternalInput"),
        "wv_t": nc.dram_tensor("wv_t", [128, 8 * GD], BF16, kind="ExternalInput"),
        "wo_t": nc.dram_tensor("wo_t", [GD, D], BF16, kind="ExternalInput"),
        "bq": nc.dram_tensor("bq", [GD, 1], F32, kind="ExternalInput"),
        "bk": nc.dram_tensor("bk", [GD, 1], F32, kind="ExternalInput"),
        "bv": nc.dram_tensor("bv", [1, GD], F32, kind="ExternalInput"),
        "bo": nc.dram_tensor("bo", [1, D], F32, kind="ExternalInput"),
        "y": nc.dram_tensor("y", [S, D], BF16, kind="ExternalOutput"),
    }

    with tile.TileContext(nc) as tc:
        with tc.tile_pool(name="persist", bufs=1) as pp:
            qT = [pp.tile([128, S], BF16, tag=f"qT{i}", name=f"qT{i}") for i in range(2)]
            kT = [pp.tile([128, T], BF16, tag=f"kT{i}", name=f"kT{i}") for i in range(2)]
            vp = [pp.tile([128, NKC * VBLK], BF16, tag=f"vp{i}", name=f"vp{i}") for i in range(2)]
            wo_sb = [pp.tile([128, D], BF16, tag=f"wo{i}", name=f"wo{i}") for i in range(2)]
            bqt = [pp.tile([128, 1], F32, tag=f"bq{i}", name=f"bq{i}") for i in range(2)]
            bkt = [pp.tile([128, 1], F32, tag=f"bk{i}", name=f"bk{i}") for i in range(2)]
            for i in range(2):
                nc.scalar.dma_start(wo_sb[i][:], dr["wo_t"][i * 128 : (i + 1) * 128, :])
                nc.scalar.dma_start(bqt[i][:], dr["bq"][i * 128 : (i + 1) * 128, :])
                nc.scalar.dma_start(bkt[i][:], dr["bk"][i * 128 : (i + 1) * 128, :])
            bv_sb = pp.tile([1, GD], F32, tag="bv")
            bo_sb = pp.tile([1, D], F32, tag="bo")
            nc.scalar.dma_start(bv_sb[:], dr["bv"][:])
            nc.scalar.dma_start(bo_sb[:], dr["bo"][:])
            bv_bc = pp.tile([128, GD], F32, tag="bvbc")
            bo_bc = pp.tile([128, D], F32, tag="bobc")
            nc.gpsimd.partition_broadcast(bv_bc[:], bv_sb[:])
            nc.gpsimd.partition_broadcast(bo_bc[:], bo_sb[:])
            # ones columns of v' (value columns overwritten later)
            for i in range(2):
                nc.vector.memset(vp[i][:], 1.0)

            if "b" not in parts:
                for i in range(2):
                    nc.vector.memset(qT[i][:], 0.0)
                    nc.vector.memset(kT[i][:], 0.0)
            pers = (qT, kT, vp, wo_sb, bqt, bkt, bv_bc, bo_bc)
            for _rep in range(reps):
                _emit_body(nc, tc, tile, mybir, dr, pers, parts)
    nc.compile()
    return nc


def _get_program(reps=1, parts="baevdy"):
    key = f"nc{reps}_{parts}"
    if key not in _prog_cache:
        _prog_cache[key] = _build_program(reps, parts)
    return _prog_cache[key]


def _host_inputs(x, sink_tokens, wq, bq, wk, bk, wv, bv, wo, bo):
    import ml_dtypes

    f = np.float32
    bf = ml_dtypes.bfloat16
    x = np.asarray(x, f)
    sink = np.asarray(sink_tokens, f)[0]            # [NS, D]
    wq, wk, wv, wo = (np.asarray(a, f) for a in (wq, wk, wv, wo))
    bq, bk, bv, bo = (np.asarray(a, f) for a in (bq, bk, bv, bo))
    sc = np.float32(1.0 / np.sqrt(HD))

    def wblocks(w):      # [GD, D] slice -> [128, 8*GD] (dd blocks side by side)
        return np.ascontiguousarray(
            w.T.reshape(8, 128, GD).transpose(1, 0, 2).reshape(128, 8 * GD)
        ).astype(bf)

    in_maps = []
    for core in range(NCORES):
        b, g = core // 4, core % 4
        xs = np.concatenate([x[b], sink], axis=0)   # sinks LAST
        xT = np.ascontiguousarray(xs.T).astype(bf)
        sl = slice(g * GD, (g + 1) * GD)
        in_maps.append({
            "xT": xT,
            "wq_t": wblocks(wq[sl] * sc),
            "wk_t": wblocks(wk[sl]),
            "wv_t": wblocks(wv[sl]),
            "wo_t": np.ascontiguousarray(wo[:, sl].T).astype(bf),
            "bq": (bq[sl] * sc).reshape(GD, 1).copy(),
            "bk": bk[sl].reshape(GD, 1).copy(),
            "bv": bv[sl].reshape(1, GD).copy(),
            "bo": (bo if g == 0 else np.zeros_like(bo)).reshape(1, D).copy(),
        })
    return in_maps


def kernel(x, sink_tokens, wq, bq, wk, bk, wv, bv, wo, bo):
    from concourse.bass_utils import run_bass_kernel_spmd

    nc = _get_program()
    in_maps = _host_inputs(x, sink_tokens, wq, bq, wk, bk, wv, bv, wo, bo)
    res = None
    last_exc = None
    for attempt in range(3):
        try:
            res = run_bass_kernel_spmd(nc, in_maps, core_ids=list(range(NCORES)))
            break
        except Exception as e:  # transient NRT/axon failures: retry
            last_exc = e
            import time as _time
            _time.sleep(2.0 * (attempt + 1))
    if res is None:
        raise last_exc
    y = np.zeros((B, S, D), np.float64)
    for core in range(NCORES):
        y[core // 4] += res.results[core]["y"]
    return y.astype(np.float32)
